# revision 50
# baseline (speedup 1.0000x reference)
"""Trainium2 Bass kernel for nn_CoAttentionLayer2 (dense_transformer).

Sharding: pure data parallel — batch B=8 mapped 1:1 onto 8 NeuronCores.
Each core runs the full co-attention layer for one batch element; no
collectives. Weights are replicated (folded with the LN affine on host).

Schedule (all-f16 PE, exp-only ACT, hot start):

  Inputs/outputs stream as f16 (half the HBM traffic of f32); all
  matmuls are f16.  rstd = 1/sqrt(var+eps) runs on the DVE via the
  fast-inverse-sqrt bit trick (+2 Newton steps), so the scalar engine
  only ever evaluates Exp/Identity — a single ACT table-set load,
  forced off the critical path by a dummy exp at t0.  A few
  dependency-free warmup matmuls bridge PE idle in the prologue so the
  DVFS clock ramps before real work.

  prologue (minimal): LN(kv0-3) -> K(0,0) -> LN(q0-3) -> Q(0,0), then
  attention starts (~25us).  Everything else — LN(kv4-7)/LN(q4-7)
  (transposes on the pm PSUM ring), K/Q/V projections, o-proj — runs
  as deadline-ordered fillers inside the attention steps, front-loaded
  while the exp stream is short.  attn@v emission is gated on the V
  copyback having been emitted (emission order == PE execution order).

  attention: 64 steps of (dots pair -> exp -> attn@v pair); head pairs
  share the PE via disjoint row groups (the second dots matmul of each
  pair overlaps the first almost completely).  attn@v accumulates per
  (head, query-chunk) into 1-bank [65,512] PSUM tiles; softmax row
  sums ride an augmented ones-column in V.  Copybacks on DVE/ACT,
  xhat on ACT before the exp stream starts and gpsimd after;
  normalize: DVE staged reciprocal + gpsimd partition_broadcast + DVE
  multiply (f16 out), interleaved with the final av of the other head.
  tail: output projection for the second token half, f16 DMA out.

PSUM budget (8 banks): dots ring 2x[128,1024] = 4, filler/transpose/
warmup ring 2x[128,512] = 2, attn@v po0/po1 1 bank each = 2.
"""

import collections
import os

import numpy as np

import concourse.bass as bass
import concourse.mybir as mybir
import concourse.tile as tile
from concourse import bacc
from concourse.bass_utils import run_bass_kernel_spmd
from concourse.masks import make_identity

P = 128
B = 8
N = 1024  # tokens (queries == keys)
D = 512  # model dim
HEADS = 8
DH = 64
INNER = 512
SCALE = DH**-0.5
EPS = 1e-5
F32 = mybir.dt.float32
F16 = mybir.dt.float16

KO = D // P  # 4 contraction tiles
JT = INNER // P  # 4 output-feature tiles (== head pairs)
TT = N // P  # 8 token tiles
IC = 2  # query/token chunks of 512
NQC = N // IC  # 512
LAG = 4  # attn@v trails dots/exp by this many steps (mid-phase)
EX_BUFS = 16  # deep ex ring so early av lateness never blocks emit_step
# pending-av threshold per phase step: drains the old chunk's attn@v
# early so its po bank frees before the next chunk's first attn@v
# (po bufs=1), without bursting more than 2 av pairs per step
AV_THRESH = [3, 2, 3, 4, 4, 4, 3, 2]
WARMUP_MM = 9  # dummy matmuls to ramp the PE clock before real work
FISR_C = 0x5F3759DF  # fast-inverse-sqrt magic (rstd on DVE, no ACT sqrt)


def _build_nc(with_bias: bool):
    nc = bacc.Bacc(
        "TRN2",
        target_bir_lowering=False,
        debug=False,
        num_devices=B,
    )

    xq_d = nc.declare_dram_parameter("xq", [N, D], F16, isOutput=False)
    xkv_d = nc.declare_dram_parameter("xkv", [N, D], F16, isOutput=False)
    wq_d = nc.declare_dram_parameter("wq", [D, INNER], F16, isOutput=False)
    wk_d = nc.declare_dram_parameter("wk", [D, INNER], F16, isOutput=False)
    wv_d = nc.declare_dram_parameter("wv", [D, INNER], F16, isOutput=False)
    wo_d = nc.declare_dram_parameter("wo", [INNER, D], F16, isOutput=False)
    if with_bias:
        bq_d = nc.declare_dram_parameter("bq", [INNER], F32, isOutput=False)
        bk_d = nc.declare_dram_parameter("bk", [INNER], F32, isOutput=False)
        bv_d = nc.declare_dram_parameter("bv", [INNER], F16, isOutput=False)
    out_d = nc.declare_dram_parameter("out", [N, D], F16, isOutput=True)
    debug = bool(int(os.environ.get("KDBG", "0")))
    if debug:
        dbg_d = {
            name: nc.declare_dram_parameter(name, shape, F16, isOutput=True)
            for name, shape in [
                ("d_xhq", [P, KO, N]),
                ("d_xhkv", [P, KO, N]),
                ("d_qt", [P, JT, N]),
                ("d_kt", [P, JT, N]),
                ("d_vg", [P, TT, HEADS, DH + 1]),
                ("d_outT", [P, JT, N]),
            ]
        }

    with tile.TileContext(nc) as tc:
        with (
            tc.tile_pool(name="singles", bufs=1) as singles,
            tc.tile_pool(name="big", bufs=1) as big,
            tc.tile_pool(name="work", bufs=3) as work,
            tc.tile_pool(name="ps", bufs=2, space="PSUM") as ps,
        ):
            eps_sb = singles.tile([P, 1], F32)
            nc.vector.memset(eps_sb, EPS)

            ident = singles.tile([P, P], F16)
            make_identity(nc, ident)

            # dummy exp: forces the exp_and_others table set to load now,
            # off the exp stream's critical path (the only ACT funcs used
            # anywhere are Exp and Identity — one table load total)
            scratch1 = singles.tile([P, 1], F32)
            nc.scalar.activation(
                out=scratch1[:],
                in_=eps_sb[:],
                func=mybir.ActivationFunctionType.Exp,
                scale=1.0,
            )

            # PE warmup: dependency-free matmuls interleaved through the
            # prologue keep the array continuously busy from t~7us so the
            # DVFS clock ramps to max before (and through) the real work
            wscr = singles.tile([P, NQC], F16)
            nc.vector.memset(wscr, 0.0)

            def warm(n):
                for _ in range(n):
                    wps = ps.tile([P, NQC], F32, tag="pm", name="warm")
                    nc.tensor.matmul(wps[:], ident[:], wscr[:], start=True, stop=True)

            # ---- input/weight DMA on the sync queue, earliest-needed first
            xts = {}  # (which, tt) -> SBUF tile of x rows

            def x_tile_dma(x_d, which, tt):
                xt = work.tile([P, D], F16, tag="ln_in", bufs=9)
                nc.sync.dma_start(out=xt[:], in_=x_d[tt * P : (tt + 1) * P, :])
                xts[(which, tt)] = xt

            def x_batch_dma(x_d, which, lo):
                xt4 = work.tile([P, 4, D], F16, tag="ln_in4", bufs=4)
                nc.sync.dma_start(
                    out=xt4[:],
                    in_=x_d[lo * P : (lo + 4) * P, :].rearrange(
                        "(t p) d -> p t d", p=P
                    ),
                )
                for i in range(4):
                    xts[(which, lo + i)] = xt4[:, i, :]

            wk_sb = singles.tile([P, KO, INNER], F16)
            wq_sb = singles.tile([P, KO, INNER], F16)
            wv_sb = singles.tile([P, KO, INNER], F16)
            wo_sb = singles.tile([P, KO, D], F16)

            x_batch_dma(xkv_d, "kv", 0)
            nc.sync.dma_start(out=wk_sb[:], in_=wk_d.rearrange("(ko p) j -> p ko j", p=P))
            x_batch_dma(xq_d, "q", 0)
            nc.sync.dma_start(out=wq_sb[:], in_=wq_d.rearrange("(ko p) j -> p ko j", p=P))
            x_batch_dma(xkv_d, "kv", 4)
            nc.sync.dma_start(out=wv_sb[:], in_=wv_d.rearrange("(ko p) j -> p ko j", p=P))
            x_batch_dma(xq_d, "q", 4)
            nc.sync.dma_start(out=wo_sb[:], in_=wo_d.rearrange("(co p) j -> p co j", p=P))

            if with_bias:
                bq_sb = singles.tile([P, JT], F32)
                bk_sb = singles.tile([P, JT], F32)
                nc.sync.dma_start(out=bq_sb[:], in_=bq_d.rearrange("(t p) -> p t", p=P))
                nc.sync.dma_start(out=bk_sb[:], in_=bk_d.rearrange("(t p) -> p t", p=P))
                bv_row = singles.tile([1, INNER], F16)
                bv_ap = bv_d.ap()
                nc.sync.dma_start(
                    out=bv_row[:],
                    in_=bass.AP(
                        tensor=bv_ap.tensor, offset=bv_ap.offset, ap=[[0, 1], [1, INNER]]
                    ),
                )
                ones_row = singles.tile([1, P], F16)
                nc.vector.memset(ones_row, 1.0)

            # ---- persistent activations ----
            xhatT_q = big.tile([P, KO, N], F16)  # [d%128, d//128, token]
            xhatT_kv = big.tile([P, KO, N], F16)
            QT = big.tile([P, JT, N], F16)  # [j%128, j//128, token]
            KT = big.tile([P, JT, N], F16)
            # [key%128, keytile, h, dh | ones] — col 64 is 1.0 so row 64
            # of po accumulates the softmax row sums
            Vg = big.tile([P, TT, HEADS, DH + 1], F16)
            outT = big.tile([P, JT, N], F16)  # [c%128, c//128, token]

            ones_pad = singles.tile([P, 1], F16)
            nc.vector.memset(ones_pad, 1.0)
            nc.vector.tensor_copy(
                out=Vg[:, :, :, DH : DH + 1],
                in_=ones_pad[:, None, None, :].to_broadcast((P, TT, HEADS, 1)),
            )

            # ---- LayerNorm: stats + rstd batched per burst of 4 tiles ----
            # rstd = exp(-0.5 * ln(var + eps)) keeps ACT in the one table set
            def ln_stats(which, tt, mv4, slot):
                stats = work.tile([P, 6], F32, tag="ln_stats", bufs=4)
                nc.vector.bn_stats(out=stats[:], in_=xts[(which, tt)])
                nc.vector.bn_aggr(out=mv4[:, slot, :], in_=stats[:])

            def ln_rstd(mv4, nburst):
                # rstd = 1/sqrt(var+eps) fully on DVE (fast-inverse-sqrt
                # seed + 2 Newton steps) — keeps ACT exp/identity-only so
                # the activation table set never reloads
                nb = nburst
                lv = work.tile([P, 5 * nb], F32, tag="ln_rstd", bufs=4)
                lvi = lv.bitcast(mybir.dt.int32)
                v, y = lv[:, 0:nb], lv[:, nb : 2 * nb]
                t, u = lv[:, 2 * nb : 3 * nb], lv[:, 3 * nb : 4 * nb]
                nmr4 = lv[:, 4 * nb : 5 * nb]
                nc.vector.tensor_scalar(
                    out=v, in0=mv4[:, 0:nb, 1], scalar1=float(EPS), scalar2=None,
                    op0=mybir.AluOpType.add,
                )
                # y = bitcast(C - (bits(v) >> 1))
                nc.vector.tensor_scalar(
                    out=lvi[:, nb : 2 * nb], in0=lvi[:, 0:nb],
                    scalar1=1, scalar2=None,
                    op0=mybir.AluOpType.arith_shift_right,
                )
                # C - h == (h ^ 0xffffffff) + (C + 1) — int-safe ops only
                # (bitwise and arith ops can't share one tensor_scalar)
                nc.vector.tensor_scalar(
                    out=lvi[:, nb : 2 * nb], in0=lvi[:, nb : 2 * nb],
                    scalar1=-1, scalar2=None,
                    op0=mybir.AluOpType.bitwise_xor,
                )
                nc.vector.tensor_scalar(
                    out=lvi[:, nb : 2 * nb], in0=lvi[:, nb : 2 * nb],
                    scalar1=FISR_C + 1, scalar2=None,
                    op0=mybir.AluOpType.add,
                )
                # Newton x2, 3 ops each: y' = ((-0.5*v*y)*y + 1.5)*y
                # (identical algebra to y *= 1.5 - 0.5*v*y^2, two fewer
                # DVE instructions on the head-critical chain)
                nc.vector.scalar_tensor_tensor(out=t, in0=v, scalar=-0.5, in1=y,
                                               op0=mybir.AluOpType.mult,
                                               op1=mybir.AluOpType.mult)
                nc.vector.tensor_tensor(out=u, in0=t, in1=y, op=mybir.AluOpType.mult)
                nc.vector.scalar_tensor_tensor(out=t, in0=u, scalar=1.5, in1=y,
                                               op0=mybir.AluOpType.add,
                                               op1=mybir.AluOpType.mult)
                nc.vector.scalar_tensor_tensor(out=u, in0=v, scalar=-0.5, in1=t,
                                               op0=mybir.AluOpType.mult,
                                               op1=mybir.AluOpType.mult)
                nc.vector.tensor_tensor(out=y, in0=u, in1=t, op=mybir.AluOpType.mult)
                nc.vector.scalar_tensor_tensor(out=y, in0=y, scalar=1.5, in1=t,
                                               op0=mybir.AluOpType.add,
                                               op1=mybir.AluOpType.mult)
                # nmr = -(mu * rstd)
                nc.vector.scalar_tensor_tensor(
                    out=nmr4,
                    in0=mv4[:, 0:nb, 0],
                    scalar=-1.0,
                    in1=y,
                    op0=mybir.AluOpType.mult,
                    op1=mybir.AluOpType.mult,
                )
                return y, nmr4

            def ln_finish(which, tt, rstd4, nmr4, slot, xhat_eng, cb_eng):
                xhat = work.tile([P, D], F16, tag="xhat", bufs=4)
                if xhat_eng == "act":
                    nc.scalar.activation(
                        out=xhat[:],
                        in_=xts[(which, tt)],
                        func=mybir.ActivationFunctionType.Identity,
                        bias=nmr4[:, slot : slot + 1],
                        scale=rstd4[:, slot : slot + 1],
                    )
                elif xhat_eng == "dve":
                    nc.vector.tensor_scalar(
                        out=xhat[:],
                        in0=xts[(which, tt)],
                        scalar1=rstd4[:, slot : slot + 1],
                        scalar2=nmr4[:, slot : slot + 1],
                        op0=mybir.AluOpType.mult,
                        op1=mybir.AluOpType.add,
                    )
                else:
                    nc.gpsimd.tensor_scalar(
                        out=xhat[:],
                        in0=xts[(which, tt)],
                        scalar1=rstd4[:, slot : slot + 1],
                        scalar2=nmr4[:, slot : slot + 1],
                        op0=mybir.AluOpType.mult,
                        op1=mybir.AluOpType.add,
                    )
                pt = ps.tile([P, D], F16, tag="pm", name="pt")
                for db in range(KO):
                    nc.tensor.transpose(
                        pt[:, db * P : (db + 1) * P], xhat[:, db * P : (db + 1) * P], ident[:]
                    )
                xhatT = xhatT_kv if which == "kv" else xhatT_q
                dst = xhatT[:, :, tt * P : (tt + 1) * P]
                src = pt[:].rearrange("p (ko t) -> p ko t", t=P)
                if cb_eng == "dve":
                    nc.vector.tensor_copy(out=dst, in_=src)
                else:
                    nc.scalar.copy(out=dst, in_=src)

            # ---- projection chunks as unit lists (for filler scheduling) ----
            def qk_units(w_sb, src, dstT, jt, ic, b_sb=None, cb_eng="dve"):
                box = {}

                def mk(ko):
                    def f():
                        if ko == 0:
                            box["pm"] = ps.tile([P, NQC], F32, tag="pm", name="pm")
                        nc.tensor.matmul(
                            box["pm"][:],
                            w_sb[:, ko, jt * P : (jt + 1) * P],
                            src[:, ko, ic * NQC : (ic + 1) * NQC],
                            start=(ko == 0),
                            stop=(ko == KO - 1),
                        )

                    return f

                def copy():
                    dst = dstT[:, jt, ic * NQC : (ic + 1) * NQC]
                    if b_sb is not None:
                        nc.vector.tensor_scalar(
                            out=dst,
                            in0=box["pm"][:],
                            scalar1=b_sb[:, jt : jt + 1],
                            scalar2=None,
                            op0=mybir.AluOpType.add,
                        )
                    elif cb_eng == "dve":
                        nc.vector.tensor_copy(out=dst, in_=box["pm"][:])
                    else:
                        nc.scalar.copy(out=dst, in_=box["pm"][:])

                return [mk(ko) for ko in range(KO)] + [copy]

            v_done = [False] * TT

            def v_units(tt, cb_eng="dve"):
                box = {}
                last = KO - 1 if not with_bias else KO

                def mk(ko):
                    def f():
                        if ko == 0:
                            box["pm"] = ps.tile([P, NQC], F32, tag="pm", name="pmv")
                        nc.tensor.matmul(
                            box["pm"][:],
                            xhatT_kv[:, ko, tt * P : (tt + 1) * P],
                            wv_sb[:, ko, :],
                            start=(ko == 0),
                            stop=(ko == last),
                        )

                    return f

                units = [mk(ko) for ko in range(KO)]

                if with_bias:

                    def bias_mm():
                        # pm += ones ⊗ bv (rank-1 bias add on the PE)
                        nc.tensor.matmul(
                            box["pm"][:],
                            ones_row[0:1, 0:P],
                            bv_row[0:1, :],
                            start=False,
                            stop=True,
                        )

                    units.append(bias_mm)

                def copy():
                    dst = Vg[:, tt, :, 0:DH]
                    src = box["pm"][:].rearrange("p (h d) -> p h d", d=DH)
                    if cb_eng == "dve":
                        nc.vector.tensor_copy(out=dst, in_=src)
                    else:
                        nc.scalar.copy(out=dst, in_=src)
                    v_done[tt] = True

                units.append(copy)
                return units

            # ---- attention ----
            pending_av = collections.deque()

            def do_av(pair, ic, kt, ex, po):
                for hh in range(2):
                    h = 2 * pair + hh
                    nc.tensor.matmul(
                        po[hh][:, :],
                        Vg[:, kt, h, :],
                        ex[:, hh * NQC : (hh + 1) * NQC],
                        start=(kt == 0),
                        stop=(kt == TT - 1),
                    )
                    if kt == TT - 1:
                        # normalize hh overlaps the other head's final av
                        normalize_h(pair, ic, po, hh)

            def normalize_h(pair, ic, po, hh):
                if True:
                    hb = hh * DH
                    # custom-DVE reciprocal can't read PSUM — stage rowsums
                    rtmp = work.tile([1, 2 * NQC], F32, tag="rectmp", bufs=4)
                    rs, rec = rtmp[:, 0:NQC], rtmp[:, NQC : 2 * NQC]
                    nc.vector.tensor_copy(out=rs, in_=po[hh][DH : DH + 1, :])
                    nc.vector.reciprocal_approx_fast(out=rec, in_=rs)
                    recB = work.tile([DH, NQC], F32, tag="recB", bufs=4)
                    nc.gpsimd.partition_broadcast(recB[:], rec[:])
                    nc.vector.tensor_tensor(
                        out=outT[hb : hb + DH, pair, ic * NQC : (ic + 1) * NQC],
                        in0=po[hh][0:DH, :],
                        in1=recB[:],
                        op=mybir.AluOpType.mult,
                    )

            def emit_step(pair, ic, kt, po):
                pd = ps.tile([P, N], F32, tag="big", name="pd")
                for hh in range(2):
                    nc.tensor.matmul(
                        pd[:, hh * NQC : (hh + 1) * NQC],
                        KT[hh * DH : (hh + 1) * DH, pair, kt * P : (kt + 1) * P],
                        QT[hh * DH : (hh + 1) * DH, pair, ic * NQC : (ic + 1) * NQC],
                        start=True,
                        stop=True,
                        tile_position=(hh * DH, 0),
                    )
                ex = work.tile([P, N], F16, tag="expT", bufs=EX_BUFS)
                nc.scalar.activation(
                    out=ex[:],
                    in_=pd[:],
                    func=mybir.ActivationFunctionType.Exp,
                    scale=SCALE,
                )
                pending_av.append((pair, ic, kt, ex, po))

            # ---- output projection ----
            ost = big.tile([P, 4, D], F32)  # staged co{0,1} partials, tt4-7

            def o_pass1_units(tt):
                box = {}

                def mk(co):
                    def f():
                        if co == 0:
                            box["pm"] = ps.tile([P, NQC], F32, tag="pm", name="pmo1")
                        nc.tensor.matmul(
                            box["pm"][:],
                            outT[:, co, tt * P : (tt + 1) * P],
                            wo_sb[:, co, :],
                            start=(co == 0),
                            stop=(co == 1),
                        )

                    return f

                def copy():
                    nc.vector.tensor_copy(out=ost[:, tt - 4, :], in_=box["pm"][:])

                return [mk(0), mk(1), copy]

            def o_pass2_units(tt):
                box = {}

                def mk(co):
                    def f():
                        if co == 2:
                            box["pm"] = ps.tile([P, NQC], F32, tag="pm", name="pmo2")
                        nc.tensor.matmul(
                            box["pm"][:],
                            outT[:, co, tt * P : (tt + 1) * P],
                            wo_sb[:, co, :],
                            start=(co == 2),
                            stop=(co == 3),
                        )

                    return f

                def add_dma():
                    ot = work.tile([P, D], F16, tag="out", bufs=3)
                    nc.vector.tensor_tensor(
                        out=ot[:], in0=box["pm"][:], in1=ost[:, tt - 4, :],
                        op=mybir.AluOpType.add,
                    )
                    nc.sync.dma_start(out=out_d[tt * P : (tt + 1) * P, :], in_=ot[:])

                return [mk(2), mk(3), add_dma]

            def o_units(tt, cb_eng="dve"):
                box = {}

                def mk(co):
                    def f():
                        if co == 0:
                            box["pm"] = ps.tile([P, NQC], F32, tag="pm", name="pmo")
                        nc.tensor.matmul(
                            box["pm"][:],
                            outT[:, co, tt * P : (tt + 1) * P],
                            wo_sb[:, co, :],
                            start=(co == 0),
                            stop=(co == KO - 1),
                        )

                    return f

                def copy_dma():
                    ot = work.tile([P, D], F16, tag="out", bufs=3)
                    if cb_eng == "dve":
                        nc.vector.tensor_copy(out=ot[:], in_=box["pm"][:])
                    else:
                        nc.scalar.copy(out=ot[:], in_=box["pm"][:])
                    nc.sync.dma_start(out=out_d[tt * P : (tt + 1) * P, :], in_=ot[:])

                return [mk(co) for co in range(KO)] + [copy_dma]

            # ================= emission =================
            # minimal prologue: only what the first dots needs.
            # burst 1: LN kv0-3 (xhat on ACT — exp stream not running yet)
            warm(8)
            mv4_a = work.tile([P, 4, 2], F32, tag="ln_mv", bufs=4)
            for tt in range(4):
                ln_stats("kv", tt, mv4_a, tt)
            rstd_a, nmr_a = ln_rstd(mv4_a, 4)
            for tt in range(4):
                ln_finish("kv", tt, rstd_a, nmr_a, tt, "act" if tt % 2 == 0 else "dve", "dve")
            warm(1)
            bkq = bk_sb if with_bias else None
            bqq = bq_sb if with_bias else None
            emit_chunk = lambda units: [u() for u in units]
            emit_chunk(qk_units(wk_sb, xhatT_kv, KT, 0, 0, b_sb=bkq, cb_eng="dve"))
            warm(2)
            # burst 2: LN q0-3
            mv4_b = work.tile([P, 4, 2], F32, tag="ln_mv", bufs=4)
            for tt in range(4):
                ln_stats("q", tt, mv4_b, tt)
            rstd_b, nmr_b = ln_rstd(mv4_b, 4)
            for tt in range(4):
                ln_finish("q", tt, rstd_b, nmr_b, tt, "act" if tt % 2 == 0 else "dve", "act" if tt < 2 else "dve")
            warm(1)
            emit_chunk(qk_units(wq_sb, xhatT_q, QT, 0, 0, b_sb=bqq, cb_eng="dve"))

            # everything else is a filler unit, ordered by deadline:
            #   K(0,1) by step 4; V(0..3) by steps ~4-8; LN kv4-7 feeds
            #   V(4..7) by steps ~8-12; K(1,*), Q(0,1), LN q4-7, Q(1,*),
            #   K(2..3,*), Q(2..3,*); o-proj late
            fillers = collections.deque()

            def ln_burst_units(which, lo, xhat_eng):
                mv4 = work.tile([P, 4, 2], F32, tag="ln_mv", bufs=4)
                units = []
                for i in range(4):
                    units.append(lambda i=i: ln_stats(which, lo + i, mv4, i))
                box = {}

                def rstd():
                    box["r"] = ln_rstd(mv4, 4)

                units.append(rstd)
                for i in range(4):
                    units.append(
                        lambda i=i: ln_finish(
                            which, lo + i, box["r"][0], box["r"][1], i, xhat_eng, "dve"
                        )
                    )
                return units

            # ordering constraints: ln_kv4 before K(*,1)/V(4-7); ln_q4
            # before Q(*,1); K(0,1) before dots kt4 (g4); Q(0,1) before
            # dots ic1 (g8); V(kt) before its (gated) av; K/Q(p,*) before
            # pair p's chunks (g16p)
            fillers.extend(ln_burst_units("kv", 4, "gpsimd"))
            fillers.extend(qk_units(wk_sb, xhatT_kv, KT, 0, 1, b_sb=bkq))
            fillers.extend(v_units(0))
            fillers.extend(qk_units(wk_sb, xhatT_kv, KT, 1, 0, b_sb=bkq))
            fillers.extend(qk_units(wq_sb, xhatT_q, QT, 1, 0, b_sb=bqq))
            fillers.extend(qk_units(wk_sb, xhatT_kv, KT, 1, 1, b_sb=bkq))
            fillers.extend(v_units(1))
            fillers.extend(v_units(2))
            fillers.extend(v_units(3))
            fillers.extend(ln_burst_units("q", 4, "gpsimd"))
            fillers.extend(v_units(4))
            fillers.extend(v_units(5))
            fillers.extend(qk_units(wq_sb, xhatT_q, QT, 0, 1, b_sb=bqq))
            fillers.extend(v_units(6))
            fillers.extend(v_units(7))
            fillers.extend(qk_units(wq_sb, xhatT_q, QT, 1, 1, b_sb=bqq))
            fillers.extend(qk_units(wk_sb, xhatT_kv, KT, 2, 0, b_sb=bkq))
            fillers.extend(qk_units(wq_sb, xhatT_q, QT, 2, 0, b_sb=bqq))
            fillers.extend(qk_units(wk_sb, xhatT_kv, KT, 3, 0, b_sb=bkq))
            fillers.extend(qk_units(wq_sb, xhatT_q, QT, 3, 0, b_sb=bqq))
            fillers.extend(qk_units(wk_sb, xhatT_kv, KT, 2, 1, b_sb=bkq))
            fillers.extend(qk_units(wk_sb, xhatT_kv, KT, 3, 1, b_sb=bkq))
            fillers.extend(qk_units(wq_sb, xhatT_q, QT, 2, 1, b_sb=bqq))
            fillers.extend(qk_units(wq_sb, xhatT_q, QT, 3, 1, b_sb=bqq))

            for tt in range(4, TT):
                fillers.extend(o_pass1_units(tt))

            late = collections.deque()  # o_proj for token tiles 0-3
            for tt in range(4):
                late.extend(o_units(tt))

            # filler quota per global step: front-load hard while the exp
            # stream is short (PE-paced), taper once fillers thin out
            def quota(gstep):
                if gstep < 8:
                    return 6
                if gstep < 16:
                    return 4
                if gstep < 32:
                    return 2
                if gstep < 48:
                    return 1
                return 2

            gstep = 0
            CH = [(0, 0), (1, 0), (0, 1), (1, 1), (2, 0), (3, 0), (2, 1), (3, 1)]
            for ci, (pair, ic) in enumerate(CH):
                if True:
                    po = (
                        ps.tile([DH + 1, NQC], F32, tag="po0", bufs=1, name="po0"),
                        ps.tile([DH + 1, NQC], F32, tag="po1", bufs=1, name="po1"),
                    )
                    for kt in range(TT):
                        assert len(pending_av) < EX_BUFS - 1, "ex ring too small"
                        emit_step(pair, ic, kt, po)
                        thresh = AV_THRESH[kt] if gstep >= 8 else LAG
                        # av(kt) must be EMITTED after V(kt)'s copyback —
                        # emission order is execution order on the PE
                        while len(pending_av) > thresh and v_done[pending_av[0][2]]:
                            do_av(*pending_av.popleft())
                        for _ in range(quota(gstep)):
                            if fillers:
                                fillers.popleft()()
                        # o_proj(tt0-3) needs every pair's ic0 normalize;
                        # (3,0) is chunk 5 — start late fillers mid-chunk-6
                        if (ci == 6 and kt >= 4) or ci == 7:
                            for _ in range(2):
                                if late:
                                    late.popleft()()
                        gstep += 1
            while fillers:
                fillers.popleft()()
            while pending_av:
                do_av(*pending_av.popleft())
            while late:
                late.popleft()()

            # tail: co{2,3} + staged-partial add for token tiles 4-7.
            # All co=2 matmuls (pair2-ic1 outT, ready long ago) go first so
            # the in-order PE chews them while DVE/gpsimd finish the last
            # normalize; only then the co=3 matmuls + adds (which wait on it)
            tail_units = [o_pass2_units(tt) for tt in range(4, TT)]
            # software-pipeline at pm-ring depth 2: never more than two
            # open accumulation groups, so no WAR wait lands ahead of the
            # matmul that would satisfy it in the in-order PE queue
            tail_units[0][0]()
            tail_units[1][0]()
            for i in range(4):
                tail_units[i][1]()  # mk(3) — waits the final normalize
                tail_units[i][2]()  # add + DMA
                if i + 2 < 4:
                    tail_units[i + 2][0]()  # next mk(2)

            if debug:
                for name, t in [
                    ("d_xhq", xhatT_q),
                    ("d_xhkv", xhatT_kv),
                    ("d_qt", QT),
                    ("d_kt", KT),
                    ("d_vg", Vg),
                    ("d_outT", outT),
                ]:
                    nc.sync.dma_start(out=dbg_d[name].ap(), in_=t[:])

    nc.compile()
    return nc


_NC_CACHE = {}


def _get_nc(with_bias: bool):
    key = ("nc", with_bias)
    if key not in _NC_CACHE:
        _NC_CACHE[key] = _build_nc(with_bias)
    return _NC_CACHE[key]


def _prep_in_maps(query, keyvalue, Wq, Wkv, Wo, gamma, beta):
    query = np.ascontiguousarray(query, dtype=np.float32)
    keyvalue = np.ascontiguousarray(keyvalue, dtype=np.float32)
    Wq = np.asarray(Wq, dtype=np.float32)
    Wkv = np.asarray(Wkv, dtype=np.float32)
    Wo = np.ascontiguousarray(Wo, dtype=np.float32)
    gamma = np.asarray(gamma, dtype=np.float32)
    beta = np.asarray(beta, dtype=np.float32)

    # fold LN affine into the projections: (xhat*g + b) @ W = xhat @ (g[:,None]*W) + b @ W
    wq_eff = np.ascontiguousarray((gamma[:, None] * Wq).astype(np.float16))
    wkv_eff = gamma[:, None] * Wkv
    bq = np.ascontiguousarray(beta @ Wq)
    bkv = beta @ Wkv
    wk_eff = np.ascontiguousarray(wkv_eff[:, :INNER].astype(np.float16))
    wv_eff = np.ascontiguousarray(wkv_eff[:, INNER:].astype(np.float16))
    bk = np.ascontiguousarray(bkv[:INNER])
    bv = np.ascontiguousarray(bkv[INNER:])
    wo_eff = np.ascontiguousarray(Wo.astype(np.float16))

    with_bias = bool(np.abs(bq).max() > 0 or np.abs(bk).max() > 0 or np.abs(bv).max() > 0)

    maps = []
    for b in range(B):
        m = dict(
            xq=np.ascontiguousarray(query[b].astype(np.float16)),
            xkv=np.ascontiguousarray(keyvalue[b].astype(np.float16)),
            wq=wq_eff,
            wk=wk_eff,
            wv=wv_eff,
            wo=wo_eff,
        )
        if with_bias:
            m["bq"] = bq
            m["bk"] = bk
            m["bv"] = bv.astype(np.float16)
        maps.append(m)
    return maps, with_bias


def run_sharded(inputs, **spmd_kwargs):
    """Run the SPMD kernel; returns (stacked output [B, N, D], BassKernelResults)."""
    in_maps, with_bias = _prep_in_maps(**inputs)
    nc = _get_nc(with_bias)
    r = run_bass_kernel_spmd(nc, in_maps, core_ids=list(range(B)), **spmd_kwargs)
    out = np.stack([r.results[b]["out"] for b in range(B)], axis=0).astype(np.float32)
    return out, r


def kernel(query, keyvalue, Wq, Wkv, Wo, gamma, beta):
    out, _ = run_sharded(
        dict(query=query, keyvalue=keyvalue, Wq=Wq, Wkv=Wkv, Wo=Wo, gamma=gamma, beta=beta)
    )
    return out
